# revision 10
# baseline (speedup 1.0000x reference)
"""NT-Xent loss kernel for Trainium2 (8 NeuronCores, Bass/Tile).

Strategy (see sharding hint): rows of the 2Nx2N similarity matrix are
sharded across the 8 cores.  Host-side we only do data marshalling:
z = concat(z1, z2) and each core receives np.roll(z, -1024*c, axis=0)
so that the SPMD kernel always works on rows [0, 1024) of its rotated
view (row permutation leaves each row's logsumexp unchanged, maps the
diagonal to the diagonal, and maps the positive-pair column to the
static range [4096, 5120)).

On-device per core:
  1. DMA the full rotated z [8192, 256] fp32.
  2. Row norms via fused DVE tensor_tensor_reduce (z*z, sum) ->
     ACT sqrt -> max(eps) -> DVE reciprocal.
  3. Normalize (fp32 -> bf16) with per-partition tensor_scalar mul.
  4. PE-transpose (128x128 blocks) into znT [2x128, 8192] bf16.
  5. For its 8 row-tiles x 16 col-chunks: 512-wide bf16 matmuls
     (K=256 accumulated in PSUM fp32), then one ACT Exp(scale=10)
     over [128, 2048] PSUM with accum_out -> fused row sums.
  6. lse = Ln(rowsum - exp(diag)), pos = 10 * <zn_i, zn_{i+4096}>.
  7. Output [128, 2] per-partition partial sums of (lse, pos).

Host combines: loss = (sum(lse) - sum(pos)) / 8192.
"""

import sys

if "/opt/trn_rl_repo" not in sys.path:
    sys.path.insert(0, "/opt/trn_rl_repo")

import numpy as np

import concourse.bass as bass
import concourse.bacc as bacc
import concourse.mybir as mybir
import concourse.tile as tile
from concourse.masks import make_identity

P = 128
D = 256
M = 8192            # 2N rows
NCORES = 8
NT = M // P         # 64 row tiles of the full z
IT = (M // NCORES) // P   # 8 row tiles owned per core
TEMP_INV = 10.0     # 1 / temperature
EPS = 1e-8
F32 = mybir.dt.float32
BF16 = mybir.dt.bfloat16
CHUNK = 2048        # columns of sim handled per PSUM tile / ACT pass
NSUB = CHUNK // 512

_nc_cache = None


def _build():
    nc = bacc.Bacc(None, target_bir_lowering=False)
    z = nc.dram_tensor("z", [M, D], F32, kind="ExternalInput")
    out = nc.dram_tensor("out", [P, 2], F32, kind="ExternalOutput")

    AF = mybir.ActivationFunctionType
    ALU = mybir.AluOpType

    with (
        tile.TileContext(nc) as tc,
        tc.tile_pool(name="big", bufs=1) as big,
        tc.tile_pool(name="small", bufs=1) as small,
        tc.tile_pool(name="scr", bufs=4) as scr,
        tc.tile_pool(name="zpool", bufs=24) as zpool,
    ):
        znn = big.tile([P, NT, D], BF16)     # normalized z (natural layout)
        znT = big.tile([P, 2, M], BF16)      # normalized z transposed
        # Dead output buffers: walrus only encodes ONE sync-wait per ACT
        # instruction, so every ACT op must write a never-reused subtile
        # (slot reuse would add a self-retirement wait). fp8e5 keeps them
        # small; the data is never read (only accum_out matters).
        FP8 = mybir.dt.float8e5
        sq_dead = big.tile([P, NT, D], FP8)
        exp_dead = big.tile([P, IT * (M // CHUNK), CHUNK], FP8)
        ss = small.tile([P, NT], F32)        # row norms^2 then scratch
        rn = small.tile([P, NT], F32)        # 1 / max(norm, eps)
        ident = small.tile([P, P], BF16)
        make_identity(nc, ident)

        zv = z.rearrange("(t p) d -> p t d", p=P)

        # ---- load + norms + normalize, in groups of 8 row tiles ----
        GK = 8
        for g in range(NT // GK):
            ztiles = []
            for j in range(GK):
                t = g * GK + j
                zrt = zpool.tile([P, D], F32, tag="zrt", name=f"zrt_{t}")
                nc.sync.dma_start(out=zrt, in_=zv[:, t, :])
                ztiles.append(zrt)
            for j in range(GK):
                t = g * GK + j
                nc.scalar.activation(
                    out=sq_dead[:, t, :],
                    in_=ztiles[j],
                    func=AF.Square,
                    accum_out=ss[:, t : t + 1],
                )
            sl = slice(g * GK, (g + 1) * GK)
            nc.scalar.activation(rn[:, sl], ss[:, sl], AF.Sqrt)
            nc.vector.tensor_scalar_max(rn[:, sl], rn[:, sl], EPS)
            nc.vector.reciprocal(rn[:, sl], rn[:, sl])
            for j in range(GK):
                t = g * GK + j
                nc.vector.tensor_scalar_mul(
                    znn[:, t, :], ztiles[j], rn[:, t : t + 1]
                )

        # ---- transpose znn -> znT via PE (128x128 blocks) ----
        with tc.tile_pool(name="ptp", bufs=2, space="PSUM") as ptp:
            for q in range(NT // 4):  # 4 row tiles -> 512 columns of znT
                pt = ptp.tile([P, 2, 4, P], BF16)
                for j in range(4):
                    t = q * 4 + j
                    for k in range(2):
                        nc.tensor.transpose(
                            pt[:, k, j, :], znn[:, t, k * P : (k + 1) * P], ident
                        )
                for k in range(2):
                    nc.vector.tensor_copy(
                        out=znT[:, k, q * 512 : (q + 1) * 512],
                        in_=pt[:, k].rearrange("p j c -> p (j c)"),
                    )

        # ---- main loop: sim row-block x col-chunk, fused exp row sums ----
        acc = small.tile([P, IT, M // CHUNK], F32)
        with tc.tile_pool(name="psp", bufs=2, space="PSUM") as psp:
            for i in range(IT):
                for c in range(M // CHUNK):
                    ps = psp.tile([P, CHUNK], F32)
                    for k in range(2):
                        for n in range(NSUB):
                            nc.tensor.matmul(
                                ps[:, n * 512 : (n + 1) * 512],
                                lhsT=znT[:, k, i * P : (i + 1) * P],
                                rhs=znT[
                                    :, k, c * CHUNK + n * 512 : c * CHUNK + (n + 1) * 512
                                ],
                                start=(k == 0),
                                stop=(k == 1),
                            )
                    nc.scalar.activation(
                        out=exp_dead[:, i * (M // CHUNK) + c, :],
                        in_=ps[:],
                        func=AF.Exp,
                        scale=TEMP_INV,
                        accum_out=acc[:, i, c : c + 1],
                    )

        # ---- tail: lse and pos partial sums ----
        rowsum = small.tile([P, IT], F32)
        nc.vector.reduce_sum(rowsum, acc, axis=mybir.AxisListType.X)

        dd = small.tile([P, IT], F32)   # <zn_i, zn_i> (bf16-consistent diag)
        pp = small.tile([P, IT], F32)   # 10 * <zn_i, zn_{i+4096}>
        for i in range(IT):
            scd = scr.tile([P, D], F32, tag="ttr_scr")
            nc.vector.tensor_mul(scd, znn[:, i, :], znn[:, i, :])
            nc.vector.reduce_sum(
                dd[:, i : i + 1], scd, axis=mybir.AxisListType.X
            )
            scp = scr.tile([P, D], F32, tag="ttr_scr")
            nc.vector.tensor_mul(scp, znn[:, i, :], znn[:, (M // 2) // P + i, :])
            nc.vector.reduce_sum(
                pp[:, i : i + 1], scp, axis=mybir.AxisListType.X
            )
        nc.vector.tensor_scalar_mul(pp, pp, TEMP_INV)

        ed = small.tile([P, IT], F32)
        nc.scalar.activation(ed, dd, AF.Exp, scale=TEMP_INV)
        nc.vector.tensor_sub(rowsum, rowsum, ed)
        lse = small.tile([P, IT], F32)
        nc.scalar.activation(lse, rowsum, AF.Ln)

        outs = small.tile([P, 2], F32)
        nc.vector.reduce_sum(outs[:, 0:1], lse, axis=mybir.AxisListType.X)
        nc.vector.reduce_sum(outs[:, 1:2], pp, axis=mybir.AxisListType.X)
        nc.sync.dma_start(out=out[:], in_=outs)

    nc.finalize()
    return nc


def _get_nc():
    global _nc_cache
    if _nc_cache is None:
        _nc_cache = _build()
    return _nc_cache


def _run_cores(z: np.ndarray, trace: bool = False):
    """Run the SPMD kernel on 8 cores. Returns (per-core results, perf)."""
    from concourse.bass_utils import run_bass_kernel_spmd

    nc = _get_nc()
    rows_per_core = M // NCORES
    in_maps = [
        {"z": np.ascontiguousarray(np.roll(z, -rows_per_core * c, axis=0))}
        for c in range(NCORES)
    ]
    res = run_bass_kernel_spmd(
        nc, in_maps, core_ids=list(range(NCORES)), trace=trace
    )
    return res


def kernel(z1: np.ndarray, z2: np.ndarray) -> np.ndarray:
    z = np.concatenate(
        [np.asarray(z1, np.float32), np.asarray(z2, np.float32)], axis=0
    )
    res = _run_cores(z)
    parts = np.stack([r["out"] for r in res.results]).astype(np.float64)
    lse_sum = parts[:, :, 0].sum()
    pos_sum = parts[:, :, 1].sum()
    return np.float32((lse_sum - pos_sum) / M)
